# revision 48
# baseline (speedup 1.0000x reference)
"""Trainium2 Bass kernel for nn_EncoderBlock (pre-norm transformer encoder block).

Sharding (8 cores, zero collectives):
  core c -> batch b = c//4, query-row block r = (c%4)*1024 .. +1024.
  Each core redundantly computes K/V for its batch, but ONLY over the keys the
  attention mask keeps (mask==0 keys contribute exp(-1e9)=0 in the reference,
  so they are dropped on the host and the kernel never sees them).

Per-core pipeline (all matmuls bf16, statistics/residuals fp32):
  norm1 -> (DMA-transpose via DRAM, band-pipelined) -> Q^T/K^T/V projections
  scores^T = K^T.T @ Q^T (head pairs packed in PE row groups, K=64 each)
  P^T = exp(scores/8 + padbias) on ScalarE (pad keys get -30 bias -> exp ~ 0)
  ctx^T accumulation: V matmul + concurrent ones-column matmul (PE col groups)
    -> softmax denominators ride along for free; scores/exp software-pipelined
    one step ahead of the ctx matmuls so PE and ACT overlap
  divide, W_O matmul + residual + norm2 interleaved per q-chunk, FFN last.
"""

import math
from contextlib import ExitStack

import ml_dtypes
import numpy as np

B, S, D = 2, 4096, 768
H, DK, DFF = 12, 64, 3072
KD = D // 128        # 6 k-tiles over d_model
FT = DFF // 128      # 24 tiles over d_ff
Q = 1024             # query rows per core
QT = Q // 128        # 8 query sub-tiles
QC = 2               # q chunks of 512
NCORES = 8
EPS = 1e-6
VAR_SCALE = float(D) / float(D - 1)  # torch.std is unbiased (ddof=1)


def _bands(ntiles, band):
    out = []
    t = 0
    while t < ntiles:
        out.append((t, min(band, ntiles - t)))
        t += band
    return out


def _build(KT, SAFE=None):
    import concourse.bass as bass
    import concourse.mybir as mybir
    import concourse.tile as tile
    from concourse import bacc
    from concourse.bass import ds, ts

    NK = KT * 128
    if SAFE is None:
        SAFE = KT - 2   # tiles < SAFE are guaranteed all-kept (pads are a suffix)
    f32 = mybir.dt.float32
    bf16 = mybir.dt.bfloat16
    i32 = mybir.dt.int32
    AF = mybir.ActivationFunctionType
    OP = mybir.AluOpType

    nc = bacc.Bacc()

    xq_d = nc.dram_tensor("xq", [Q, D], f32, kind="ExternalInput")
    xk_d = nc.dram_tensor("xk", [NK, D], bf16, kind="ExternalInput")
    km_d = nc.dram_tensor("kmask", [NK], i32, kind="ExternalInput")
    wq_d = nc.dram_tensor("wqT", [D, D], bf16, kind="ExternalInput")
    wk_d = nc.dram_tensor("wkT", [D, D], bf16, kind="ExternalInput")
    wv_d = nc.dram_tensor("wvT", [D, D], bf16, kind="ExternalInput")
    wo_d = nc.dram_tensor("woT", [D, D], bf16, kind="ExternalInput")
    w1_d = nc.dram_tensor("w1T", [D, DFF], bf16, kind="ExternalInput")
    w2_d = nc.dram_tensor("w2T", [DFF, D], bf16, kind="ExternalInput")
    bq_d = nc.dram_tensor("bq", [D], f32, kind="ExternalInput")
    bk_d = nc.dram_tensor("bk", [D], f32, kind="ExternalInput")
    bv_d = nc.dram_tensor("bv", [D], f32, kind="ExternalInput")
    bo_d = nc.dram_tensor("bo", [D], f32, kind="ExternalInput")
    b1_d = nc.dram_tensor("b1", [DFF], f32, kind="ExternalInput")
    b2_d = nc.dram_tensor("b2", [D], f32, kind="ExternalInput")
    a1_d = nc.dram_tensor("a1", [1], f32, kind="ExternalInput")
    g1_d = nc.dram_tensor("g1", [1], f32, kind="ExternalInput")
    a2_d = nc.dram_tensor("a2", [1], f32, kind="ExternalInput")
    g2_d = nc.dram_tensor("g2", [1], f32, kind="ExternalInput")
    out_d = nc.dram_tensor("out", [Q, D], f32, kind="ExternalOutput")

    def norm_tile(spool, xt, a_b, g_b, out_t):
        # out = alpha * (x - mean) / (std_unbiased + eps) + beta, reduced over D
        st = spool.tile([128, 3, 6], f32, tag="bnst")
        for g in range(3):
            nc.vector.bn_stats(st[:, g, :], xt[:, ts(g, 256)])
        mv = spool.tile([128, 2], f32, tag="bnmv")
        nc.vector.bn_aggr(mv, st)
        rp = spool.tile([128, 1], f32, tag="rp")
        nc.scalar.activation(rp, mv[:, 1:2], AF.Sqrt, bias=0.0, scale=VAR_SCALE)
        nc.vector.tensor_scalar_add(rp, rp, EPS)
        nc.vector.reciprocal(rp, rp)
        nc.vector.tensor_tensor(rp, rp, a_b, OP.mult)
        cb = spool.tile([128, 1], f32, tag="cb")
        nc.vector.tensor_tensor(cb, mv[:, 0:1], rp, OP.mult)
        nc.vector.tensor_tensor(cb, g_b, cb, OP.subtract)
        nc.vector.tensor_scalar(out_t, xt, rp, cb, OP.mult, OP.add)

    with tile.TileContext(nc) as tc, ExitStack() as ctx:
        const = ctx.enter_context(tc.tile_pool(name="const", bufs=1))
        dram = ctx.enter_context(tc.tile_pool(name="dram", bufs=1, space="DRAM"))

        # --- broadcast scalars alpha/beta -> [128,1]
        scal = {}
        for name, d_t in (("a1", a1_d), ("g1", g1_d), ("a2", a2_d), ("g2", g2_d)):
            t = const.tile([128, 1], f32, tag=f"sc_{name}")
            nc.gpsimd.dma_start(out=t, in_=d_t[:].to_broadcast((128, 1)))
            scal[name] = t

        # --- per-partition bias stripes
        bqp = const.tile([128, KD], f32, tag="bqp")
        bkp = const.tile([128, KD], f32, tag="bkp")
        b1p = const.tile([128, FT], f32, tag="b1p")

        # --- free-dim biases broadcast [D] -> [128, D] (step-0 partition on a
        # DRAM source is legal)
        ones_col = const.tile([128, 1], bf16, tag="ones_col")
        nc.vector.memset(ones_col, 1.0)
        ident = const.tile([128, 128], bf16, tag="ident")
        from concourse.masks import make_identity
        make_identity(nc, ident)

        bvb = const.tile([128, D], f32, tag="bvb")
        bob = const.tile([128, D], f32, tag="bob")
        b2b = const.tile([128, D], f32, tag="b2b")
        for d_t, dst in ((bv_d, bvb), (bo_d, bob), (b2_d, b2b)):
            src = d_t[:]
            bcast = bass.AP(tensor=src.tensor, offset=src.offset,
                            ap=[[0, 128], [1, D]])
            nc.gpsimd.dma_start(out=dst, in_=bcast)

        # --- pad-mask bias: (mask-1)*30 -> 0 for kept keys, -30 for pads
        kmi = const.tile([128, KT], i32, tag="kmi")
        kmf = const.tile([128, KT], f32, tag="kmf")
        padb = const.tile([128, KT], f32, tag="padb")

        # --- long-lived activations (pool releases are LIFO-stacked)
        cTp = ctx.enter_context(tc.tile_pool(name="cTp", bufs=1))
        cT = cTp.tile([128, KD, Q], bf16, tag="cT")
        woTp = ctx.enter_context(tc.tile_pool(name="woTp", bufs=1))
        woT_sb = woTp.tile([128, KD, D], bf16, tag="woT")
        x1p = ctx.enter_context(tc.tile_pool(name="x1p", bufs=1))
        x1 = x1p.tile([128, QT, D], f32, tag="x1")
        h2Tp = ctx.enter_context(tc.tile_pool(name="h2Tp", bufs=1))
        h2T = h2Tp.tile([128, KD, Q], bf16, tag="h2T")
        # first third of W1, loaded during attention so ff1 starts immediately
        w1ap = ctx.enter_context(tc.tile_pool(name="w1ap", bufs=1))
        w1a = w1ap.tile([128, KD, 1024], bf16, tag="w1a")

        qkv_cm = tc.tile_pool(name="qkvp", bufs=1)   # lives A..D
        qkvp = qkv_cm.__enter__()
        kT = qkvp.tile([128, KD, NK], bf16, tag="kT")
        qT = qkvp.tile([128, KD, Q], bf16, tag="qT")
        vv = qkvp.tile([128, KT, D], bf16, tag="vv")

        hk_dram = dram.tile([NK, D], bf16, tag="hk_dram")
        hq_dram = dram.tile([Q, D], bf16, tag="hq_dram")
        h2_dram = dram.tile([Q, D], bf16, tag="h2_dram")

        # ========== Phase A/B/C: norm1 + transpose + QKV, band-pipelined =====
        with tc.tile_pool(name="normA", bufs=2) as npool, \
             tc.tile_pool(name="xtp", bufs=4) as xtp, \
             tc.tile_pool(name="stats", bufs=8) as spool, \
             tc.tile_pool(name="hband", bufs=2) as hbp, \
             tc.tile_pool(name="wqkv", bufs=1) as wp, \
             tc.tile_pool(name="psC", bufs=4, space="PSUM") as pp:
            # issue band-0 x loads before the weight DMAs so the first norms
            # are not queued behind 3.5MB of weights
            b0 = []
            t0_, nt_ = _bands(KT, 4)[0]
            for t in range(t0_, t0_ + nt_):
                xt = xtp.tile([128, D], bf16, tag="xt")
                nc.sync.dma_start(xt, xk_d[ts(t, 128), :])
                b0.append(xt)

            wqs = wp.tile([128, KD, D], bf16, tag="wqs")
            wks = wp.tile([128, KD, D], bf16, tag="wks")
            wvs = wp.tile([128, KD, D], bf16, tag="wvs")
            # wk first (first K^T matmuls need it), wq last (Q bands are last)
            for k in range(KD):
                nc.sync.dma_start(wks[:, k, :], wk_d[ts(k, 128), :])
            for k in range(KD):
                nc.sync.dma_start(wvs[:, k, :], wv_d[ts(k, 128), :])
            for k in range(KD):
                nc.sync.dma_start(wqs[:, k, :], wq_d[ts(k, 128), :])

            # K-side bands: norm -> DRAM -> transpose -> K^T + V matmuls
            for bi, (t0, nt) in enumerate(_bands(KT, 4)):
                w = nt * 128
                for t in range(t0, t0 + nt):
                    if bi == 0:
                        xt = b0[t - t0]
                    else:
                        xt = xtp.tile([128, D], bf16, tag="xt")
                        nc.sync.dma_start(xt, xk_d[ts(t, 128), :])
                    ht = npool.tile([128, D], bf16, tag="ht")
                    norm_tile(spool, xt, scal["a1"], scal["g1"], ht)
                    nc.sync.dma_start(hk_dram[ts(t, 128), :], ht)
                hb = hbp.tile([128, KD, 512], bf16, tag="hb")
                for j in range(KD):
                    nc.sync.dma_start_transpose(
                        hb[:, j, :w], hk_dram[ds(t0 * 128, w), ts(j, 128)])
                # K^T for this band
                for j in range(KD):
                    ps = pp.tile([128, 512], f32, tag="psc")
                    for k in range(KD):
                        nc.tensor.matmul(ps[:, :w], wks[:, k, ts(j, 128)],
                                         hb[:, k, :w],
                                         start=(k == 0), stop=(k == KD - 1))
                    nc.vector.tensor_scalar_add(kT[:, j, ds(t0 * 128, w)],
                                                ps[:, :w], bkp[:, j:j + 1])
                # V for this band
                for t in range(t0, t0 + nt):
                    loc = (t - t0) * 128
                    for hh in range(2):
                        ps = pp.tile([128, 512], f32, tag="psc")
                        for k in range(KD):
                            nc.tensor.matmul(ps[:, :384],
                                             hb[:, k, ds(loc, 128)],
                                             wvs[:, k, ts(hh, 384)],
                                             start=(k == 0), stop=(k == KD - 1))
                        nc.vector.tensor_tensor(vv[:, t, ts(hh, 384)],
                                                ps[:, :384], bvb[:, ts(hh, 384)],
                                                OP.add)

            # Q-side bands
            for t0, nt in _bands(QT, 4):
                w = nt * 128
                for t in range(t0, t0 + nt):
                    xt = xtp.tile([128, D], f32, tag="xtq")
                    nc.sync.dma_start(xt, xq_d[ts(t, 128), :])
                    ht = npool.tile([128, D], bf16, tag="ht")
                    norm_tile(spool, xt, scal["a1"], scal["g1"], ht)
                    nc.sync.dma_start(hq_dram[ts(t, 128), :], ht)
                hb = hbp.tile([128, KD, 512], bf16, tag="hb")
                for j in range(KD):
                    nc.sync.dma_start_transpose(
                        hb[:, j, :w], hq_dram[ds(t0 * 128, w), ts(j, 128)])
                for j in range(KD):
                    ps = pp.tile([128, 512], f32, tag="psc")
                    for k in range(KD):
                        nc.tensor.matmul(ps[:, :w], wqs[:, k, ts(j, 128)],
                                         hb[:, k, :w],
                                         start=(k == 0), stop=(k == KD - 1))
                    nc.vector.tensor_scalar_add(qT[:, j, ds(t0 * 128, w)],
                                                ps[:, :w], bqp[:, j:j + 1])

        # ========== Phase D/E/F: attention + W_O + norm2, per q-chunk ========
        with tc.tile_pool(name="psS", bufs=2, space="PSUM") as psS, \
             tc.tile_pool(name="psCx", bufs=4, space="PSUM") as psCx, \
             tc.tile_pool(name="ptp", bufs=3) as ptp, \
             tc.tile_pool(name="asm", bufs=2) as asm, \
             tc.tile_pool(name="normB", bufs=3) as npool2, \
             tc.tile_pool(name="stats2", bufs=8) as spool2, \
             tc.tile_pool(name="xqb", bufs=4) as xqbp:

            def scores_exp(c, p, kt):
                pss = psS.tile([128, 1024], f32, tag="pss")
                nc.tensor.matmul(pss[:, 0:512], kT[0:64, p, ts(kt, 128)],
                                 qT[0:64, p, ts(c, 512)], start=True, stop=True)
                nc.tensor.matmul(pss[:, 512:1024], kT[64:128, p, ts(kt, 128)],
                                 qT[64:128, p, ts(c, 512)], start=True, stop=True)
                pt = ptp.tile([128, 1024], bf16, tag="pt")
                nc.scalar.activation(pt, pss, AF.Exp,
                                     bias=padb[:, kt:kt + 1], scale=0.125)
                return pt

            def divide_out(c, p, pc0, pc1):
                for hh, pc in ((0, pc0), (1, pc1)):
                    rr = asm.tile([1, 512], f32, tag="rr")
                    nc.vector.reciprocal(rr, pc[64:65, :])
                    rb = asm.tile([64, 512], f32, tag="rb")
                    nc.gpsimd.partition_broadcast(rb, rr)
                    nc.vector.tensor_tensor(cT[ds(hh * 64, 64), p, ts(c, 512)],
                                            pc[0:64, :], rb, OP.mult)

            pending = None
            for c in range(QC):
                # residual tiles for this chunk's W_O, loaded under attention
                xbs = []
                for st_ in range(4):
                    g = c * 4 + st_
                    xb = xqbp.tile([128, D], f32, tag="xb")
                    nc.sync.dma_start(xb, xq_d[ts(g, 128), :])
                    nc.vector.tensor_tensor(xb, xb, bob, OP.add)
                    xbs.append(xb)
                for p in range(KD):
                    pc0 = psCx.tile([128, 512], f32, tag="pc")
                    pc1 = psCx.tile([128, 512], f32, tag="pc")
                    pt_next = scores_exp(c, p, 0)
                    for kt in range(KT):
                        pt = pt_next
                        if kt + 1 < KT:
                            pt_next = scores_exp(c, p, kt + 1)
                        st, sp = (kt == 0), (kt == KT - 1)
                        nc.tensor.matmul(pc0[0:64, :],
                                         vv[:, kt, ds(2 * p * 64, 64)],
                                         pt[:, 0:512], start=st, stop=sp)
                        nc.tensor.matmul(pc0[64:65, :], ones_col,
                                         pt[:, 0:512], start=st, stop=sp)
                        nc.tensor.matmul(pc1[0:64, :],
                                         vv[:, kt, ds((2 * p + 1) * 64, 64)],
                                         pt[:, 512:1024], start=st, stop=sp)
                        nc.tensor.matmul(pc1[64:65, :], ones_col,
                                         pt[:, 512:1024], start=st, stop=sp)
                    if pending is not None:
                        divide_out(*pending)
                    pending = (c, p, pc0, pc1)

                divide_out(*pending)
                pending = None

                # W_O + residual for this chunk (psum slots shared with psCx)
                for st_ in range(4):
                    g = c * 4 + st_
                    xb = xbs[st_]
                    for hh in range(2):
                        ps = psCx.tile([128, 512], f32, tag="pc")
                        for j in range(KD):
                            nc.tensor.matmul(ps[:, :384], cT[:, j, ts(g, 128)],
                                             woT_sb[:, j, ts(hh, 384)],
                                             start=(j == 0), stop=(j == KD - 1))
                        nc.vector.tensor_tensor(x1[:, g, ts(hh, 384)],
                                                ps[:, :384], xb[:, ts(hh, 384)],
                                                OP.add)

                # norm2 + transpose for this chunk
                for st_ in range(4):
                    g = c * 4 + st_
                    ht = npool2.tile([128, D], bf16, tag="h2t")
                    norm_tile(spool2, x1[:, g, :], scal["a2"], scal["g2"], ht)
                    nc.sync.dma_start(h2_dram[ts(g, 128), :], ht)
                for j in range(KD):
                    nc.sync.dma_start_transpose(
                        h2T[:, j, ts(c, 512)],
                        h2_dram[ds(c * 512, 512), ts(j, 128)])

                if c == 0:
                    for k in range(KD):
                        nc.sync.dma_start(w1a[:, k, :],
                                          w1_d[ts(k, 128), 0:1024])

        qkv_cm.__exit__(None, None, None)  # free kT/qT/vv

        # ================= Phase G: FFN + residual =================
        wff = ctx.enter_context(tc.tile_pool(name="wff", bufs=1))
        w1b = wff.tile([128, KD, 2048], bf16, tag="w1b")
        for k in range(KD):
            for h3_ in range(2):
                nc.sync.dma_start(
                    w1b[:, k, ds(h3_ * 1024, 1024)],
                    w1_d[ts(k, 128), ds(1024 + h3_ * 1024, 1024)])
        w2s = wff.tile([128, FT, D], bf16, tag="w2s")
        for k in range(FT):
            nc.sync.dma_start(w2s[:, k, :], w2_d[ts(k, 128), :])

        pg = psCx
        with tc.tile_pool(name="h3p", bufs=1) as h3p, \
             tc.tile_pool(name="outp", bufs=3) as outp:
            for c in range(QC):
                h3 = h3p.tile([128, FT, 512], bf16, tag="h3")
                for f in range(FT):
                    ps = pg.tile([128, 512], f32, tag="pc")
                    for k in range(KD):
                        w1sl = (w1a[:, k, ts(f, 128)] if f < 8 else
                                w1b[:, k, ds((f - 8) * 128, 128)])
                        nc.tensor.matmul(ps, w1sl,
                                         h2T[:, k, ts(c, 512)],
                                         start=(k == 0), stop=(k == KD - 1))
                    nc.scalar.activation(h3[:, f, :], ps, AF.Relu,
                                         bias=b1p[:, f:f + 1], scale=1.0)
                for st_ in range(4):
                    g = c * 4 + st_
                    ot = outp.tile([128, D], f32, tag="ot")
                    for hh in range(2):
                        ps = pg.tile([128, 512], f32, tag="pc")
                        for k in range(FT):
                            nc.tensor.matmul(ps[:, :384], h3[:, k, ts(st_, 128)],
                                             w2s[:, k, ts(hh, 384)],
                                             start=(k == 0), stop=(k == FT - 1))
                        nc.vector.tensor_tensor(ot[:, ts(hh, 384)], ps[:, :384],
                                                x1[:, g, ts(hh, 384)], OP.add)
                    nc.vector.tensor_tensor(ot, ot, b2b, OP.add)
                    nc.sync.dma_start(out_d[ts(g, 128), :], ot)

        psCx_cm.__exit__(None, None, None)
        psD_cm.__exit__(None, None, None)

    nc.finalize()
    return nc


def _prep_inputs(inputs):
    bf = ml_dtypes.bfloat16
    x = np.asarray(inputs["x"], np.float32)
    mask = np.asarray(inputs["mask"], np.int32).reshape(B, S)

    kept = [np.nonzero(mask[b])[0] for b in range(B)]
    nk_max = max(len(kept[0]), len(kept[1]))
    KT = max(2, int(math.ceil(nk_max / 128.0)))
    SAFE = min(len(kept[0]), len(kept[1])) // 128
    NK = KT * 128

    xk = []
    km = []
    for b in range(B):
        n = len(kept[b])
        xkb = np.zeros((NK, D), np.float32)
        xkb[:n] = x[b][kept[b]]
        if n < NK:
            # pad rows get real data (not zeros) so ln(var) in the norm stays
            # finite; their attention contribution is killed by the -30 bias
            xkb[n:] = xkb[0]
        xk.append(np.ascontiguousarray(xkb.astype(bf)))
        kmb = np.zeros(NK, np.int32)
        kmb[:n] = 1
        km.append(kmb)

    def w_t(name):
        return np.ascontiguousarray(
            np.asarray(inputs[name], np.float32).T.astype(bf))

    shared = {
        "wqT": w_t("wq"), "wkT": w_t("wk"), "wvT": w_t("wv"), "woT": w_t("wo"),
        "w1T": w_t("w1"), "w2T": w_t("w2"),
        "bq": np.asarray(inputs["bq"], np.float32),
        "bk": np.asarray(inputs["bk"], np.float32),
        "bv": np.asarray(inputs["bv"], np.float32),
        "bo": np.asarray(inputs["bo"], np.float32),
        "b1": np.asarray(inputs["b1"], np.float32),
        "b2": np.asarray(inputs["b2"], np.float32),
        "a1": np.asarray(inputs["alpha1"], np.float32).reshape(1),
        "g1": np.asarray(inputs["beta1"], np.float32).reshape(1),
        "a2": np.asarray(inputs["alpha2"], np.float32).reshape(1),
        "g2": np.asarray(inputs["beta2"], np.float32).reshape(1),
    }

    in_maps = []
    for c in range(NCORES):
        b, r = c // 4, (c % 4) * Q
        m = dict(shared)
        m["xq"] = np.ascontiguousarray(x[b, r:r + Q])
        m["xk"] = xk[b]
        m["kmask"] = km[b]
        in_maps.append(m)
    return KT, SAFE, in_maps


def kernel(**inputs):
    from concourse.bass_utils import run_bass_kernel_spmd

    KT, SAFE, in_maps = _prep_inputs(inputs)
    nc = _build(KT, SAFE)
    res = run_bass_kernel_spmd(nc, in_maps, core_ids=list(range(NCORES)))
    out = np.empty((B, S, D), np.float32)
    for c in range(NCORES):
        b, r = c // 4, (c % 4) * Q
        out[b, r:r + Q] = res.results[c]["out"]
    return out


if __name__ == "__main__":
    rng = np.random.default_rng(0)
    demo = {
        "x": rng.standard_normal((B, S, D), dtype=np.float32),
        "mask": rng.integers(0, 2, (B, 1, 1, S)).astype(np.int32),
        "wq": rng.standard_normal((D, D), dtype=np.float32) * 0.02,
        "bq": np.zeros(D, np.float32),
        "wk": rng.standard_normal((D, D), dtype=np.float32) * 0.02,
        "bk": np.zeros(D, np.float32),
        "wv": rng.standard_normal((D, D), dtype=np.float32) * 0.02,
        "bv": np.zeros(D, np.float32),
        "wo": rng.standard_normal((D, D), dtype=np.float32) * 0.02,
        "bo": np.zeros(D, np.float32),
        "w1": rng.standard_normal((DFF, D), dtype=np.float32) * 0.02,
        "b1": np.zeros(DFF, np.float32),
        "w2": rng.standard_normal((D, DFF), dtype=np.float32) * 0.02,
        "b2": np.zeros(D, np.float32),
        "alpha1": np.ones(1, np.float32), "beta1": np.ones(1, np.float32),
        "alpha2": np.ones(1, np.float32), "beta2": np.ones(1, np.float32),
    }
    out = kernel(**demo)
    print("out", out.shape, out.dtype, float(np.abs(out).mean()))
